# revision 20
# baseline (speedup 1.0000x reference)
"""Multi-head attention + output projection (nn_AttentionBase) on 8 Trainium2
NeuronCores.

Reference computation (B=2, S=2048, E=2048, H=16, c=128, fp32):
    scores  = einsum('bqhc,bkhc->bhqk', q/sqrt(c), k)
    weights = softmax(scores + mask_bias_on_keys)
    out     = einsum('bhqk,bkhc->bqhc', weights, v) @ w_out.T

Sharding: 8 cores = (batch b: 2) x (query block of 512: 4); each core runs all
16 heads for its 512 queries against its batch's valid keys, then the full
output projection for its rows; no inter-core reduction, host concatenates.

Mask sparsity: padding mask on keys (~50%); host sorts keys/values
valid-first per batch, kernel processes only NCHE = ceil(max_valid/128) key
chunks (exp(-30000) == 0 makes skipped tails exact).  Chunks < NFULL are
valid for both batches: their exp runs with constant zero bias, batched two
chunks per ScalarE ACT over a 2-bank PSUM group.

Per-core dataflow (matmuls bf16, fp32 PSUM accumulation):
  scoresT[sk,sq] = kT.T @ qT            (per 128-key chunk, PE)
  pT = exp(scoresT * c^-0.5 [+ maskb])  (ScalarE)
  attnT[c,sq]   += v_chunk.T @ pT       (PE, accumulated over key chunks)
  l[1,sq]        = ones.T @ tree-sums   (bf16 tree on DVE/GpSimd, M=1 PE MMs)
  attn_sb[c,sq]  = attnT * (1/l)        (DVE; 32x32-transpose-spread recip +
                                         DRAM-bounce partition broadcast)
  y[sq,e_out]    = sum_h attn_sb_h.T @ w_outT  (PE; 7 output tiles prefill
                                         their first-8-head contraction into
                                         late phase-A PE bubbles)

Perf note (measured, this device pool): ~188-200us max-core, +-5us run-to-run
jitter.  Denser schedules (full scores/attn interleave, deeper buffering,
GpSimd offload, broadcast via K=1 matmul) were all tried and measure equal or
slower: sustained engine utilization trips the core's utilization throttle
(clock drops ~20% for 12-22% of the run; see throttle_activity_1 in the NTFF
summary), so runtime sits at an energy equilibrium rather than a schedule
limit.  This burstier schedule is the empirical optimum under that cap.
GpSimd cannot access PSUM; walrus here cannot codegen InstPartitionBroadcast;
DVE rejects zero-stride (broadcast) partition APs; K=1 matmuls wedge the
device (NRT_EXEC_UNIT_UNRECOVERABLE)."""
import sys

sys.path.insert(0, "/opt/trn_rl_repo")

import math

import ml_dtypes
import numpy as np

import concourse.bass as bass
import concourse.mybir as mybir
import concourse.tile as tile

B, S, E = 2, 2048, 2048
H, C = 16, 128
SQ = 512          # queries per core
NSQT = SQ // 128   # 4 query subtiles
NNT = E // 512     # 4 output column tiles
MASK_NEG = -30000.0
BF16 = mybir.dt.bfloat16
F32 = mybir.dt.float32
NHALF_Y = 7        # output tiles whose first-half contraction runs in phase A


_WAIT_LIMIT = 1


def _split_excess_waits(nc, limit=_WAIT_LIMIT):
    for f in nc.m.functions:
        for bb in f.blocks:
            new = []
            changed = False
            for inst in bb.instructions:
                si = inst.sync_info
                if si is not None and len(si.on_wait) > limit:
                    waits = list(si.on_wait)
                    excess, keep = waits[:-limit], waits[-limit:]
                    for k in range(0, len(excess), limit):
                        nop = mybir.InstNoOp(
                            name=f"{inst.name}-wsplit{k}",
                            sync_info=mybir.SyncInfo(
                                on_wait=excess[k:k + limit], on_update=[]
                            ),
                            bass_nofuse=True,
                            engine=inst.engine,
                        )
                        new.append(nop)
                    inst.sync_info = mybir.SyncInfo(
                        on_wait=keep, on_update=list(si.on_update)
                    )
                    changed = True
                new.append(inst)
            if changed:
                bb.instructions = new


def _build_program(nfull, nche):
    nc = bass.Bass()
    nk = nche * 128
    qT = nc.declare_dram_parameter("qT", [H, C, SQ], BF16, isOutput=False)
    kT = nc.declare_dram_parameter("kT", [H, C, nk], BF16, isOutput=False)
    v = nc.declare_dram_parameter("v", [H, 128, nche, C], BF16, isOutput=False)
    wT = nc.declare_dram_parameter("wT", [E, E], BF16, isOutput=False)
    maskb = nc.declare_dram_parameter("maskb", [128, nche], F32, isOutput=False)
    y = nc.declare_dram_parameter("y", [SQ, E], F32, isOutput=True)

    scale = 1.0 / math.sqrt(C)

    groups = []
    for g in range(nfull // 2):
        groups.append((2 * g, 2, True))
    if nfull % 2:
        groups.append((nfull - 1, 1, True))
    for j in range(nfull, nche):
        groups.append((j, 1, False))

    with tile.TileContext(nc) as tc:
        with (
            tc.tile_pool(name="consts", bufs=1) as consts,
            tc.tile_pool(name="wpool", bufs=1) as wpool,
            tc.tile_pool(name="attn_all", bufs=1) as attn_all,
            tc.tile_pool(name="kv", bufs=3) as kv,
            tc.tile_pool(name="pt", bufs=10) as ptpool,
            tc.tile_pool(name="spt", bufs=4) as sptpool,
            tc.tile_pool(name="small", bufs=4) as small,
            tc.tile_pool(name="lbc", bufs=8) as lbc,
            tc.tile_pool(name="ldram", bufs=3, space="DRAM") as ldram,
            tc.tile_pool(name="yout", bufs=3) as yout,
            tc.tile_pool(name="ypart", bufs=1) as ypart,
            tc.tile_pool(name="psS", bufs=2, space="PSUM") as psS,
            tc.tile_pool(name="psA", bufs=3, space="PSUM") as psA,
            tc.tile_pool(name="psL", bufs=1, space="PSUM") as psL,
        ):
            ones = consts.tile([128, 1], BF16)
            nc.vector.memset(ones, 1.0)
            maskb_sb = consts.tile([128, nche], F32)
            nc.sync.dma_start(maskb_sb, maskb[:, :])
            warm = consts.tile([1, 1], F32, tag="warm")
            nc.scalar.activation(warm, ones[0:1, 0:1],
                                 mybir.ActivationFunctionType.Exp)

            w_sb = wpool.tile([128, E // 128, E], BF16)
            attn_tiles = [attn_all.tile([128, SQ], BF16, tag=f"a{h}",
                                        name=f"attn{h}") for h in range(H)]

            pending = []
            yraw_tiles = {}
            for h in range(H):
                ktall = kv.tile([128, nk], BF16, tag="kt")
                qt = kv.tile([128, SQ], BF16, tag="qt")
                if h < 2:
                    step = (nche + 3) // 4 * 128
                    nc.sync.dma_start(ktall[:, :step], kT[h][:, :step])
                    nc.sync.dma_start(qt, qT[h])
                    for o in range(step, nk, step):
                        e = min(o + step, nk)
                        nc.sync.dma_start(ktall[:, o:e], kT[h][:, o:e])
                else:
                    nc.sync.dma_start(ktall, kT[h])
                    nc.sync.dma_start(qt, qT[h])
                vt = kv.tile([128, nche, C], BF16, tag="vt")
                nc.sync.dma_start(vt, v[h])
                nc.gpsimd.dma_start(w_sb[:, h, :], wT[h * 128:(h + 1) * 128, :])

                ps_at = psA.tile([128, SQ], F32, tag="at")

                pt_slices = []
                for gi, (c0, n, zb) in enumerate(groups):
                    ps_g = psS.tile([128, n * SQ], F32)
                    for jj in range(n):
                        nc.tensor.matmul(
                            ps_g[:, jj * SQ:(jj + 1) * SQ],
                            lhsT=ktall[:, (c0 + jj) * 128:(c0 + jj + 1) * 128],
                            rhs=qt,
                            start=True, stop=True,
                        )
                    pt_g = ptpool.tile([128, n * SQ], BF16)
                    bias = 0.0 if zb else maskb_sb[:, c0:c0 + 1]
                    nc.scalar.activation(
                        pt_g, ps_g, mybir.ActivationFunctionType.Exp,
                        bias=bias, scale=scale,
                    )
                    for jj in range(n):
                        pt_slices.append(pt_g[:, jj * SQ:(jj + 1) * SQ])

                for j in range(nche):
                    nc.tensor.matmul(
                        ps_at, lhsT=vt[:, j, :], rhs=pt_slices[j],
                        start=(j == 0), stop=(j == nche - 1),
                    )

                def tree(slices, who, gps_l0):
                    level = list(slices)
                    li = 0
                    while len(level) > 1:
                        nxt = []
                        for i in range(0, len(level) - 1, 2):
                            t = sptpool.tile([128, SQ], BF16,
                                             tag=f"s{who}{li}{i}")
                            eng = (nc.gpsimd if (li == 0 and i < gps_l0)
                                   else nc.vector)
                            eng.tensor_add(t, level[i], level[i + 1])
                            nxt.append(t)
                        if len(level) % 2:
                            nxt.append(level[-1])
                        level = nxt
                        li += 1
                    return level[0]
                half = min(4, nche - 1) if nche > 1 else 1
                s_halves = [tree(pt_slices[:half], "L", 4)]
                if nche > half:
                    s_halves.append(tree(pt_slices[half:], "R", 2))

                def finish_head(h=h, ps_at=ps_at, s_halves=s_halves):
                    ps_l = psL.tile([32, SQ], F32, tag="ly")
                    for si, s in enumerate(s_halves):
                        nc.tensor.matmul(ps_l[0:1, :], lhsT=ones, rhs=s,
                                         start=(si == 0),
                                         stop=(si == len(s_halves) - 1))
                    t1 = small.tile([32, SQ], F32, tag="t1")
                    nc.vector.transpose(t1, ps_l)
                    rt = small.tile([32, SQ], F32, tag="rt")
                    nc.vector.reciprocal(rt[:, ::32], t1[:, ::32])
                    t2 = small.tile([32, SQ], F32, tag="t2")
                    nc.vector.transpose(t2, rt)
                    ld = ldram.tile([1, SQ], F32)
                    nc.sync.dma_start(ld, t2[0:1, :])
                    lb = lbc.tile([128, SQ], F32)
                    nc.sync.dma_start(
                        lb,
                        bass.AP(tensor=ld.tensor, offset=ld.offset,
                                ap=[[0, 128]] + list(ld.ap[1:])),
                    )
                    nc.vector.tensor_mul(attn_tiles[h], ps_at, lb)

                pending.append(finish_head)
                if len(pending) > 1:
                    pending.pop(0)()

                if h >= H - NHALF_Y:
                    g = h - (H - NHALF_Y)
                    i, n = divmod(g, NNT)
                    ecn = H // 2
                    ps_hy = psL.tile([128, 512], F32, tag="ly")
                    for ec in range(ecn):
                        nc.tensor.matmul(
                            ps_hy,
                            lhsT=attn_tiles[ec][:, i * 128:(i + 1) * 128],
                            rhs=w_sb[:, ec, n * 512:(n + 1) * 512],
                            start=(ec == 0), stop=(ec == ecn - 1),
                        )
                    yr = ypart.tile([128, 512], F32, tag=f"yr{g}")
                    nc.vector.tensor_copy(yr, ps_hy)
                    yraw_tiles[g] = (yr, ecn)
            for p in pending:
                p()

            order = [g for g in range(NSQT * NNT) if g not in yraw_tiles] + \
                    [g for g in range(NSQT * NNT) if g in yraw_tiles]
            for gi_b, g in enumerate(order):
                if True:
                    i, n = divmod(g, NNT)
                    pool = psA if gi_b % 2 == 0 else psL
                    tag = "at" if pool is psA else "ly"
                    ps_y = pool.tile([128, 512], F32, tag=tag)
                    ec0 = yraw_tiles[g][1] if g in yraw_tiles else 0
                    for ec in range(ec0, H):
                        nc.tensor.matmul(
                            ps_y,
                            lhsT=attn_tiles[ec][:, i * 128:(i + 1) * 128],
                            rhs=w_sb[:, ec, n * 512:(n + 1) * 512],
                            start=(ec == ec0), stop=(ec == H - 1),
                        )
                    yt = yout.tile([128, 512], F32)
                    if gi_b >= NSQT * NNT - 2:
                        for hf in range(2):
                            sl = slice(hf * 256, (hf + 1) * 256)
                            if g in yraw_tiles:
                                nc.vector.tensor_add(
                                    yt[:, sl], ps_y[:, sl],
                                    yraw_tiles[g][0][:, sl])
                            else:
                                nc.scalar.copy(yt[:, sl], ps_y[:, sl])
                            nc.sync.dma_start(
                                y[i * 128:(i + 1) * 128,
                                  n * 512 + hf * 256:n * 512 + (hf + 1) * 256],
                                yt[:, sl],
                            )
                    else:
                        if g in yraw_tiles:
                            nc.vector.tensor_add(yt, ps_y, yraw_tiles[g][0])
                        else:
                            nc.scalar.copy(yt, ps_y)
                        nc.sync.dma_start(
                            y[i * 128:(i + 1) * 128, n * 512:(n + 1) * 512],
                            yt,
                        )

    _split_excess_waits(nc)
    return nc


_PROGRAMS = {}


def _get_program(nfull, nche):
    key = (nfull, nche)
    if key not in _PROGRAMS:
        _PROGRAMS[key] = _build_program(nfull, nche)
    return _PROGRAMS[key]


def _make_in_maps(keys, values, queries, attention_mask, w_out):
    bf = ml_dtypes.bfloat16
    wT_host = np.ascontiguousarray(w_out.astype(bf).T)

    nv = attention_mask.sum(axis=1).astype(np.int64)
    nfull = int(nv.min()) // 128
    nche = max(1, int(-(-int(nv.max()) // 128)))
    nk = nche * 128

    per_batch = []
    for b in range(B):
        order = np.argsort(~attention_mask[b], kind="stable")[:nk]
        kb = keys[b][order].astype(bf).reshape(nk, H, C)
        kT_host = np.ascontiguousarray(kb.transpose(1, 2, 0))
        vb = values[b][order].astype(bf).reshape(nche, 128, H, C)
        v_host = np.ascontiguousarray(vb.transpose(2, 1, 0, 3))
        mb = np.where(attention_mask[b][order], 0.0, MASK_NEG).astype(np.float32)
        maskb_host = np.ascontiguousarray(mb.reshape(nche, 128).T)
        per_batch.append((kT_host, v_host, maskb_host))

    in_maps = []
    for core in range(8):
        b = core // 4
        q0 = (core % 4) * SQ
        qb = queries[b, q0:q0 + SQ].astype(bf).reshape(SQ, H, C)
        qT_host = np.ascontiguousarray(qb.transpose(1, 2, 0))
        kT_host, v_host, maskb_host = per_batch[b]
        in_maps.append({
            "qT": qT_host,
            "kT": kT_host,
            "v": v_host,
            "wT": wT_host,
            "maskb": maskb_host,
        })
    return in_maps, nfull, nche


def _run(inputs, trace=False, trace_cores=None):
    from concourse.bass_utils import run_bass_kernel_spmd

    in_maps, nfull, nche = _make_in_maps(**inputs)
    nc = _get_program(nfull, nche)
    res = run_bass_kernel_spmd(
        nc, in_maps, core_ids=list(range(8)),
        trace=trace, trace_cores=trace_cores,
    )
    out = np.empty((B, S, E), dtype=np.float32)
    for core in range(8):
        b = core // 4
        q0 = (core % 4) * SQ
        out[b, q0:q0 + SQ, :] = res.results[core]["y"]
    return out, res


def kernel(keys, values, queries, attention_mask, w_out):
    out, _ = _run(dict(
        keys=np.asarray(keys), values=np.asarray(values),
        queries=np.asarray(queries),
        attention_mask=np.asarray(attention_mask),
        w_out=np.asarray(w_out),
    ))
    return out


# revision 21
# speedup vs baseline: 1.0187x; 1.0187x over previous
"""Multi-head attention + output projection (nn_AttentionBase) on 8 Trainium2
NeuronCores.

Reference computation (B=2, S=2048, E=2048, H=16, c=128, fp32):
    scores  = einsum('bqhc,bkhc->bhqk', q/sqrt(c), k)
    weights = softmax(scores + mask_bias_on_keys)
    out     = einsum('bhqk,bkhc->bqhc', weights, v) @ w_out.T

Sharding: 8 cores = (batch b: 2) x (query block of 512: 4); each core runs all
16 heads for its 512 queries against its batch's valid keys, then the full
output projection for its rows; no inter-core reduction, host concatenates.

Mask sparsity: padding mask on keys (~50%); host sorts keys/values
valid-first per batch, kernel processes only NCHE = ceil(max_valid/128) key
chunks (exp(-30000) == 0 makes skipped tails exact).  Chunks < NFULL are
valid for both batches: their exp runs with constant zero bias, batched two
chunks per ScalarE ACT over a 2-bank PSUM group.

Per-core dataflow (matmuls bf16, fp32 PSUM accumulation):
  scoresT[sk,sq] = kT.T @ qT            (per 128-key chunk, PE)
  pT = exp(scoresT * c^-0.5 [+ maskb])  (ScalarE)
  attnT[c,sq]   += v_chunk.T @ pT       (PE, accumulated over key chunks)
  l[1,sq]        = ones.T @ tree-sums   (bf16 tree on DVE/GpSimd, M=1 PE MMs)
  attn_sb[c,sq]  = attnT * (1/l)        (DVE; 32x32-transpose-spread recip +
                                         DRAM-bounce partition broadcast)
  y[sq,e_out]    = sum_h attn_sb_h.T @ w_outT  (PE; 7 output tiles prefill
                                         their first-8-head contraction into
                                         late phase-A PE bubbles)

Perf note (measured, this device pool): ~188-200us max-core, +-5us run-to-run
jitter.  Denser schedules (full scores/attn interleave, deeper buffering,
GpSimd offload, broadcast via K=1 matmul) were all tried and measure equal or
slower: sustained engine utilization trips the core's utilization throttle
(clock drops ~20% for 12-22% of the run; see throttle_activity_1 in the NTFF
summary), so runtime sits at an energy equilibrium rather than a schedule
limit.  This burstier schedule is the empirical optimum under that cap.
GpSimd cannot access PSUM; walrus here cannot codegen InstPartitionBroadcast;
DVE rejects zero-stride (broadcast) partition APs; K=1 matmuls wedge the
device (NRT_EXEC_UNIT_UNRECOVERABLE)."""
import sys

sys.path.insert(0, "/opt/trn_rl_repo")

import math

import ml_dtypes
import numpy as np

import concourse.bass as bass
import concourse.mybir as mybir
import concourse.tile as tile

B, S, E = 2, 2048, 2048
H, C = 16, 128
SQ = 512          # queries per core
NSQT = SQ // 128   # 4 query subtiles
NNT = E // 512     # 4 output column tiles
MASK_NEG = -30000.0
BF16 = mybir.dt.bfloat16
F32 = mybir.dt.float32
NHALF_Y = 7        # output tiles whose first-half contraction runs in phase A


_WAIT_LIMIT = 1


def _split_excess_waits(nc, limit=_WAIT_LIMIT):
    for f in nc.m.functions:
        for bb in f.blocks:
            new = []
            changed = False
            for inst in bb.instructions:
                si = inst.sync_info
                if si is not None and len(si.on_wait) > limit:
                    waits = list(si.on_wait)
                    excess, keep = waits[:-limit], waits[-limit:]
                    for k in range(0, len(excess), limit):
                        nop = mybir.InstNoOp(
                            name=f"{inst.name}-wsplit{k}",
                            sync_info=mybir.SyncInfo(
                                on_wait=excess[k:k + limit], on_update=[]
                            ),
                            bass_nofuse=True,
                            engine=inst.engine,
                        )
                        new.append(nop)
                    inst.sync_info = mybir.SyncInfo(
                        on_wait=keep, on_update=list(si.on_update)
                    )
                    changed = True
                new.append(inst)
            if changed:
                bb.instructions = new


def _build_program(nfull, nche):
    nc = bass.Bass()
    nk = nche * 128
    qT = nc.declare_dram_parameter("qT", [H, C, SQ], BF16, isOutput=False)
    kT = nc.declare_dram_parameter("kT", [H, C, nk], BF16, isOutput=False)
    v = nc.declare_dram_parameter("v", [H, 128, nche, C], BF16, isOutput=False)
    wT = nc.declare_dram_parameter("wT", [E, E], BF16, isOutput=False)
    maskb = nc.declare_dram_parameter("maskb", [128, nche], F32, isOutput=False)
    y = nc.declare_dram_parameter("y", [SQ, E], F32, isOutput=True)

    scale = 1.0 / math.sqrt(C)

    groups = []
    for g in range(nfull // 2):
        groups.append((2 * g, 2, True))
    if nfull % 2:
        groups.append((nfull - 1, 1, True))
    for j in range(nfull, nche):
        groups.append((j, 1, False))

    with tile.TileContext(nc) as tc:
        with (
            tc.tile_pool(name="consts", bufs=1) as consts,
            tc.tile_pool(name="wpool", bufs=1) as wpool,
            tc.tile_pool(name="attn_all", bufs=1) as attn_all,
            tc.tile_pool(name="kv", bufs=3) as kv,
            tc.tile_pool(name="pt", bufs=10) as ptpool,
            tc.tile_pool(name="spt", bufs=4) as sptpool,
            tc.tile_pool(name="small", bufs=4) as small,
            tc.tile_pool(name="lbc", bufs=8) as lbc,
            tc.tile_pool(name="ldram", bufs=3, space="DRAM") as ldram,
            tc.tile_pool(name="yout", bufs=3) as yout,
            tc.tile_pool(name="ypart", bufs=1) as ypart,
            tc.tile_pool(name="psS", bufs=2, space="PSUM") as psS,
            tc.tile_pool(name="psA", bufs=3, space="PSUM") as psA,
            tc.tile_pool(name="psL", bufs=1, space="PSUM") as psL,
        ):
            ones = consts.tile([128, 1], BF16)
            nc.vector.memset(ones, 1.0)
            maskb_sb = consts.tile([128, nche], F32)
            nc.sync.dma_start(maskb_sb, maskb[:, :])
            warm = consts.tile([1, 1], F32, tag="warm")
            nc.scalar.activation(warm, ones[0:1, 0:1],
                                 mybir.ActivationFunctionType.Exp)

            w_sb = wpool.tile([128, E // 128, E], BF16)
            attn_tiles = [attn_all.tile([128, SQ], BF16, tag=f"a{h}",
                                        name=f"attn{h}") for h in range(H)]

            pending = []
            yraw_tiles = {}
            for h in range(H):
                ktall = kv.tile([128, nk], BF16, tag="kt")
                qt = kv.tile([128, SQ], BF16, tag="qt")
                if h < 2:
                    step = (nche + 3) // 4 * 128
                    nc.sync.dma_start(ktall[:, :step], kT[h][:, :step])
                    nc.sync.dma_start(qt, qT[h])
                    for o in range(step, nk, step):
                        e = min(o + step, nk)
                        nc.sync.dma_start(ktall[:, o:e], kT[h][:, o:e])
                else:
                    nc.sync.dma_start(ktall, kT[h])
                    nc.sync.dma_start(qt, qT[h])
                vt = kv.tile([128, nche, C], BF16, tag="vt")
                nc.sync.dma_start(vt, v[h])
                nc.gpsimd.dma_start(w_sb[:, h, :], wT[h * 128:(h + 1) * 128, :])

                ps_at = psA.tile([128, SQ], F32, tag="at")

                pt_slices = []
                for gi, (c0, n, zb) in enumerate(groups):
                    ps_g = psS.tile([128, n * SQ], F32)
                    for jj in range(n):
                        nc.tensor.matmul(
                            ps_g[:, jj * SQ:(jj + 1) * SQ],
                            lhsT=ktall[:, (c0 + jj) * 128:(c0 + jj + 1) * 128],
                            rhs=qt,
                            start=True, stop=True,
                        )
                    pt_g = ptpool.tile([128, n * SQ], BF16)
                    bias = 0.0 if zb else maskb_sb[:, c0:c0 + 1]
                    nc.scalar.activation(
                        pt_g, ps_g, mybir.ActivationFunctionType.Exp,
                        bias=bias, scale=scale,
                    )
                    for jj in range(n):
                        pt_slices.append(pt_g[:, jj * SQ:(jj + 1) * SQ])

                for j in range(nche):
                    nc.tensor.matmul(
                        ps_at, lhsT=vt[:, j, :], rhs=pt_slices[j],
                        start=(j == 0), stop=(j == nche - 1),
                    )

                def tree(slices, who, gps_l0):
                    level = list(slices)
                    li = 0
                    while len(level) > 1:
                        nxt = []
                        for i in range(0, len(level) - 1, 2):
                            t = sptpool.tile([128, SQ], BF16,
                                             tag=f"s{who}{li}{i}")
                            eng = (nc.gpsimd if (li == 0 and i < gps_l0)
                                   else nc.vector)
                            eng.tensor_add(t, level[i], level[i + 1])
                            nxt.append(t)
                        if len(level) % 2:
                            nxt.append(level[-1])
                        level = nxt
                        li += 1
                    return level[0]
                half = min(4, nche - 1) if nche > 1 else 1
                s_halves = [tree(pt_slices[:half], "L", 4)]
                if nche > half:
                    s_halves.append(tree(pt_slices[half:], "R", 2))

                def finish_head(h=h, ps_at=ps_at, s_halves=s_halves):
                    ps_l = psL.tile([32, SQ], F32, tag="ly")
                    for si, s in enumerate(s_halves):
                        nc.tensor.matmul(ps_l[0:1, :], lhsT=ones, rhs=s,
                                         start=(si == 0),
                                         stop=(si == len(s_halves) - 1))
                    t1 = small.tile([32, SQ], F32, tag="t1")
                    nc.vector.transpose(t1, ps_l)
                    rt = small.tile([32, SQ], F32, tag="rt")
                    nc.vector.reciprocal(rt[:, ::32], t1[:, ::32])
                    t2 = small.tile([32, SQ], F32, tag="t2")
                    nc.vector.transpose(t2, rt)
                    ld = ldram.tile([1, SQ], F32)
                    nc.sync.dma_start(ld, t2[0:1, :])
                    lb = lbc.tile([128, SQ], F32)
                    nc.sync.dma_start(
                        lb,
                        bass.AP(tensor=ld.tensor, offset=ld.offset,
                                ap=[[0, 128]] + list(ld.ap[1:])),
                    )
                    nc.vector.tensor_mul(attn_tiles[h], ps_at, lb)

                pending.append(finish_head)
                # defer the finish chain TWO heads: during pipeline ramp the
                # DVE runs ~a head behind PE, and a 1-head deferral makes the
                # l-matmuls of heads 2-8 stall PE ~1-5us each waiting on the
                # tree sums; two heads of slack absorbs the ramp.  psA keeps
                # bufs=3 so attn(h) waits mul(h-3), which lands mid-head with
                # the DVE lag - just in time.
                if len(pending) > 2:
                    pending.pop(0)()

                if h >= H - NHALF_Y:
                    g = h - (H - NHALF_Y)
                    i, n = divmod(g, NNT)
                    ecn = H // 2
                    ps_hy = psL.tile([128, 512], F32, tag="ly")
                    for ec in range(ecn):
                        nc.tensor.matmul(
                            ps_hy,
                            lhsT=attn_tiles[ec][:, i * 128:(i + 1) * 128],
                            rhs=w_sb[:, ec, n * 512:(n + 1) * 512],
                            start=(ec == 0), stop=(ec == ecn - 1),
                        )
                    yr = ypart.tile([128, 512], F32, tag=f"yr{g}")
                    nc.vector.tensor_copy(yr, ps_hy)
                    yraw_tiles[g] = (yr, ecn)
            for p in pending:
                p()

            order = [g for g in range(NSQT * NNT) if g not in yraw_tiles] + \
                    [g for g in range(NSQT * NNT) if g in yraw_tiles]
            for gi_b, g in enumerate(order):
                if True:
                    i, n = divmod(g, NNT)
                    pool = psA if gi_b % 2 == 0 else psL
                    tag = "at" if pool is psA else "ly"
                    ps_y = pool.tile([128, 512], F32, tag=tag)
                    ec0 = yraw_tiles[g][1] if g in yraw_tiles else 0
                    for ec in range(ec0, H):
                        nc.tensor.matmul(
                            ps_y,
                            lhsT=attn_tiles[ec][:, i * 128:(i + 1) * 128],
                            rhs=w_sb[:, ec, n * 512:(n + 1) * 512],
                            start=(ec == ec0), stop=(ec == H - 1),
                        )
                    yt = yout.tile([128, 512], F32)
                    if gi_b >= NSQT * NNT - 2:
                        for hf in range(2):
                            sl = slice(hf * 256, (hf + 1) * 256)
                            if g in yraw_tiles:
                                nc.vector.tensor_add(
                                    yt[:, sl], ps_y[:, sl],
                                    yraw_tiles[g][0][:, sl])
                            else:
                                nc.scalar.copy(yt[:, sl], ps_y[:, sl])
                            nc.sync.dma_start(
                                y[i * 128:(i + 1) * 128,
                                  n * 512 + hf * 256:n * 512 + (hf + 1) * 256],
                                yt[:, sl],
                            )
                    else:
                        if g in yraw_tiles:
                            nc.vector.tensor_add(yt, ps_y, yraw_tiles[g][0])
                        else:
                            nc.scalar.copy(yt, ps_y)
                        nc.sync.dma_start(
                            y[i * 128:(i + 1) * 128, n * 512:(n + 1) * 512],
                            yt,
                        )

    _split_excess_waits(nc)
    return nc


_PROGRAMS = {}


def _get_program(nfull, nche):
    key = (nfull, nche)
    if key not in _PROGRAMS:
        _PROGRAMS[key] = _build_program(nfull, nche)
    return _PROGRAMS[key]


def _make_in_maps(keys, values, queries, attention_mask, w_out):
    bf = ml_dtypes.bfloat16
    wT_host = np.ascontiguousarray(w_out.astype(bf).T)

    nv = attention_mask.sum(axis=1).astype(np.int64)
    nfull = int(nv.min()) // 128
    nche = max(1, int(-(-int(nv.max()) // 128)))
    nk = nche * 128

    per_batch = []
    for b in range(B):
        order = np.argsort(~attention_mask[b], kind="stable")[:nk]
        kb = keys[b][order].astype(bf).reshape(nk, H, C)
        kT_host = np.ascontiguousarray(kb.transpose(1, 2, 0))
        vb = values[b][order].astype(bf).reshape(nche, 128, H, C)
        v_host = np.ascontiguousarray(vb.transpose(2, 1, 0, 3))
        mb = np.where(attention_mask[b][order], 0.0, MASK_NEG).astype(np.float32)
        maskb_host = np.ascontiguousarray(mb.reshape(nche, 128).T)
        per_batch.append((kT_host, v_host, maskb_host))

    in_maps = []
    for core in range(8):
        b = core // 4
        q0 = (core % 4) * SQ
        qb = queries[b, q0:q0 + SQ].astype(bf).reshape(SQ, H, C)
        qT_host = np.ascontiguousarray(qb.transpose(1, 2, 0))
        kT_host, v_host, maskb_host = per_batch[b]
        in_maps.append({
            "qT": qT_host,
            "kT": kT_host,
            "v": v_host,
            "wT": wT_host,
            "maskb": maskb_host,
        })
    return in_maps, nfull, nche


def _run(inputs, trace=False, trace_cores=None):
    from concourse.bass_utils import run_bass_kernel_spmd

    in_maps, nfull, nche = _make_in_maps(**inputs)
    nc = _get_program(nfull, nche)
    res = run_bass_kernel_spmd(
        nc, in_maps, core_ids=list(range(8)),
        trace=trace, trace_cores=trace_cores,
    )
    out = np.empty((B, S, E), dtype=np.float32)
    for core in range(8):
        b = core // 4
        q0 = (core % 4) * SQ
        out[b, q0:q0 + SQ, :] = res.results[core]["y"]
    return out, res


def kernel(keys, values, queries, attention_mask, w_out):
    out, _ = _run(dict(
        keys=np.asarray(keys), values=np.asarray(values),
        queries=np.asarray(queries),
        attention_mask=np.asarray(attention_mask),
        w_out=np.asarray(w_out),
    ))
    return out


# revision 22
# speedup vs baseline: 1.1052x; 1.0849x over previous
"""Multi-head attention + output projection (nn_AttentionBase) on 8 Trainium2
NeuronCores.

Reference computation (B=2, S=2048, E=2048, H=16, c=128, fp32):
    scores  = einsum('bqhc,bkhc->bhqk', q/sqrt(c), k)
    weights = softmax(scores + mask_bias_on_keys)
    out     = einsum('bhqk,bkhc->bqhc', weights, v) @ w_out.T

Sharding: 8 cores = (batch b: 2) x (query block of 512: 4); each core runs all
16 heads for its 512 queries against its batch's valid keys, then the full
output projection for its rows; no inter-core reduction, host concatenates.

Mask sparsity: padding mask on keys (~50%); host sorts keys/values
valid-first per batch, kernel processes only NCHE = ceil(max_valid/128) key
chunks (exp(-30000) == 0 makes skipped tails exact).  Chunks < NFULL are
valid for both batches: their exp runs with constant zero bias, batched two
chunks per ScalarE ACT over a 2-bank PSUM group.

Per-core dataflow (matmuls bf16, fp32 PSUM accumulation):
  scoresT[sk,sq] = kT.T @ qT            (per 128-key chunk, PE)
  pT = exp(scoresT * c^-0.5 [+ maskb])  (ScalarE)
  attnT[c,sq]   += v_chunk.T @ pT       (PE, accumulated over key chunks)
  l[1,sq]        = ones.T @ tree-sums   (bf16 tree on DVE/GpSimd, M=1 PE MMs)
  attn_sb[c,sq]  = attnT * (1/l)        (DVE; 32x32-transpose-spread recip +
                                         DRAM-bounce partition broadcast)
  y[sq,e_out]    = sum_h attn_sb_h.T @ w_outT  (PE; 7 output tiles prefill
                                         their first-8-head contraction into
                                         late phase-A PE bubbles)

Perf note (measured, this device pool): ~188-200us max-core, +-5us run-to-run
jitter.  Denser schedules (full scores/attn interleave, deeper buffering,
GpSimd offload, broadcast via K=1 matmul) were all tried and measure equal or
slower: sustained engine utilization trips the core's utilization throttle
(clock drops ~20% for 12-22% of the run; see throttle_activity_1 in the NTFF
summary), so runtime sits at an energy equilibrium rather than a schedule
limit.  This burstier schedule is the empirical optimum under that cap.
GpSimd cannot access PSUM; walrus here cannot codegen InstPartitionBroadcast;
DVE rejects zero-stride (broadcast) partition APs; K=1 matmuls wedge the
device (NRT_EXEC_UNIT_UNRECOVERABLE)."""
import sys

sys.path.insert(0, "/opt/trn_rl_repo")

import math

import ml_dtypes
import numpy as np

import concourse.bass as bass
import concourse.mybir as mybir
import concourse.tile as tile

B, S, E = 2, 2048, 2048
H, C = 16, 128
SQ = 512          # queries per core
NSQT = SQ // 128   # 4 query subtiles
NNT = E // 512     # 4 output column tiles
MASK_NEG = -30000.0
BF16 = mybir.dt.bfloat16
F32 = mybir.dt.float32
NHALF_Y = 7        # output tiles whose first-half contraction runs in phase A


_WAIT_LIMIT = 1


def _split_excess_waits(nc, limit=_WAIT_LIMIT):
    for f in nc.m.functions:
        for bb in f.blocks:
            new = []
            changed = False
            for inst in bb.instructions:
                si = inst.sync_info
                if si is not None and len(si.on_wait) > limit:
                    waits = list(si.on_wait)
                    excess, keep = waits[:-limit], waits[-limit:]
                    for k in range(0, len(excess), limit):
                        nop = mybir.InstNoOp(
                            name=f"{inst.name}-wsplit{k}",
                            sync_info=mybir.SyncInfo(
                                on_wait=excess[k:k + limit], on_update=[]
                            ),
                            bass_nofuse=True,
                            engine=inst.engine,
                        )
                        new.append(nop)
                    inst.sync_info = mybir.SyncInfo(
                        on_wait=keep, on_update=list(si.on_update)
                    )
                    changed = True
                new.append(inst)
            if changed:
                bb.instructions = new


def _build_program(nfull, nche):
    nc = bass.Bass()
    nk = nche * 128
    qT = nc.declare_dram_parameter("qT", [H, C, SQ], BF16, isOutput=False)
    kT = nc.declare_dram_parameter("kT", [H, C, nk], BF16, isOutput=False)
    v = nc.declare_dram_parameter("v", [H, 128, nche, C], BF16, isOutput=False)
    wT = nc.declare_dram_parameter("wT", [E, E], BF16, isOutput=False)
    maskb = nc.declare_dram_parameter("maskb", [128, nche], F32, isOutput=False)
    y = nc.declare_dram_parameter("y", [SQ, E], F32, isOutput=True)

    scale = 1.0 / math.sqrt(C)

    groups = []
    for g in range(nfull // 2):
        groups.append((2 * g, 2, True))
    if nfull % 2:
        groups.append((nfull - 1, 1, True))
    for j in range(nfull, nche):
        groups.append((j, 1, False))

    with tile.TileContext(nc) as tc:
        with (
            tc.tile_pool(name="consts", bufs=1) as consts,
            tc.tile_pool(name="wpool", bufs=1) as wpool,
            tc.tile_pool(name="attn_all", bufs=1) as attn_all,
            tc.tile_pool(name="kv", bufs=3) as kv,
            tc.tile_pool(name="pt", bufs=10) as ptpool,
            tc.tile_pool(name="spt", bufs=4) as sptpool,
            tc.tile_pool(name="small", bufs=4) as small,
            tc.tile_pool(name="lbc", bufs=8) as lbc,
            tc.tile_pool(name="ldram", bufs=3, space="DRAM") as ldram,
            tc.tile_pool(name="yout", bufs=3) as yout,
            tc.tile_pool(name="ypart", bufs=1) as ypart,
            tc.tile_pool(name="psS", bufs=2, space="PSUM") as psS,
            tc.tile_pool(name="psA", bufs=3, space="PSUM") as psA,
            tc.tile_pool(name="psL", bufs=1, space="PSUM") as psL,
        ):
            ones = consts.tile([128, 1], BF16)
            nc.vector.memset(ones, 1.0)
            maskb_sb = consts.tile([128, nche], F32)
            nc.sync.dma_start(maskb_sb, maskb[:, :])
            warm = consts.tile([1, 1], F32, tag="warm")
            nc.scalar.activation(warm, ones[0:1, 0:1],
                                 mybir.ActivationFunctionType.Exp)

            w_sb = wpool.tile([128, E // 128, E], BF16)
            attn_tiles = [attn_all.tile([128, SQ], BF16, tag=f"a{h}",
                                        name=f"attn{h}") for h in range(H)]

            pending = []
            yraw_tiles = {}
            for h in range(H):
                ktall = kv.tile([128, nk], BF16, tag="kt")
                qt = kv.tile([128, SQ], BF16, tag="qt")
                if h < 2:
                    step = (nche + 3) // 4 * 128
                    nc.sync.dma_start(ktall[:, :step], kT[h][:, :step])
                    nc.sync.dma_start(qt, qT[h])
                    for o in range(step, nk, step):
                        e = min(o + step, nk)
                        nc.sync.dma_start(ktall[:, o:e], kT[h][:, o:e])
                else:
                    nc.sync.dma_start(ktall, kT[h])
                    nc.sync.dma_start(qt, qT[h])
                vt = kv.tile([128, nche, C], BF16, tag="vt")
                nc.sync.dma_start(vt, v[h])
                nc.gpsimd.dma_start(w_sb[:, h, :], wT[h * 128:(h + 1) * 128, :])

                ps_at = psA.tile([128, SQ], F32, tag="at")

                pt_slices = []
                for gi, (c0, n, zb) in enumerate(groups):
                    ps_g = psS.tile([128, n * SQ], F32)
                    for jj in range(n):
                        nc.tensor.matmul(
                            ps_g[:, jj * SQ:(jj + 1) * SQ],
                            lhsT=ktall[:, (c0 + jj) * 128:(c0 + jj + 1) * 128],
                            rhs=qt,
                            start=True, stop=True,
                        )
                    pt_g = ptpool.tile([128, n * SQ], BF16)
                    bias = 0.0 if zb else maskb_sb[:, c0:c0 + 1]
                    nc.scalar.activation(
                        pt_g, ps_g, mybir.ActivationFunctionType.Exp,
                        bias=bias, scale=scale,
                    )
                    for jj in range(n):
                        pt_slices.append(pt_g[:, jj * SQ:(jj + 1) * SQ])

                for j in range(nche):
                    nc.tensor.matmul(
                        ps_at, lhsT=vt[:, j, :], rhs=pt_slices[j],
                        start=(j == 0), stop=(j == nche - 1),
                    )

                def tree(slices, who, gps_l0):
                    level = list(slices)
                    li = 0
                    while len(level) > 1:
                        nxt = []
                        for i in range(0, len(level) - 1, 2):
                            t = sptpool.tile([128, SQ], BF16,
                                             tag=f"s{who}{li}{i}")
                            eng = (nc.gpsimd if (li == 0 and i < gps_l0)
                                   else nc.vector)
                            eng.tensor_add(t, level[i], level[i + 1])
                            nxt.append(t)
                        if len(level) % 2:
                            nxt.append(level[-1])
                        level = nxt
                        li += 1
                    return level[0]
                half = min(4, nche - 1) if nche > 1 else 1
                s_halves = [tree(pt_slices[:half], "L", 4)]
                if nche > half:
                    s_halves.append(tree(pt_slices[half:], "R", 2))

                def finish_head(h=h, ps_at=ps_at, s_halves=s_halves):
                    ps_l = psL.tile([32, SQ], F32, tag="ly")
                    for si, s in enumerate(s_halves):
                        nc.tensor.matmul(ps_l[0:1, :], lhsT=ones, rhs=s,
                                         start=(si == 0),
                                         stop=(si == len(s_halves) - 1))
                    t1 = small.tile([32, SQ], F32, tag="t1")
                    nc.vector.transpose(t1, ps_l)
                    rt = small.tile([32, SQ], F32, tag="rt")
                    nc.vector.reciprocal(rt[:, ::32], t1[:, ::32])
                    t2 = small.tile([32, SQ], F32, tag="t2")
                    nc.vector.transpose(t2, rt)
                    ld = ldram.tile([1, SQ], F32)
                    nc.sync.dma_start(ld, t2[0:1, :])
                    lb = lbc.tile([128, SQ], F32)
                    nc.sync.dma_start(
                        lb,
                        bass.AP(tensor=ld.tensor, offset=ld.offset,
                                ap=[[0, 128]] + list(ld.ap[1:])),
                    )
                    nc.vector.tensor_mul(attn_tiles[h], ps_at, lb)

                pending.append(finish_head)
                if len(pending) > 1:
                    pending.pop(0)()

                if h >= H - NHALF_Y:
                    g = h - (H - NHALF_Y)
                    i, n = divmod(g, NNT)
                    ecn = H // 2
                    ps_hy = psL.tile([128, 512], F32, tag="ly")
                    for ec in range(ecn):
                        nc.tensor.matmul(
                            ps_hy,
                            lhsT=attn_tiles[ec][:, i * 128:(i + 1) * 128],
                            rhs=w_sb[:, ec, n * 512:(n + 1) * 512],
                            start=(ec == 0), stop=(ec == ecn - 1),
                        )
                    yr = ypart.tile([128, 512], F32, tag=f"yr{g}")
                    nc.vector.tensor_copy(yr, ps_hy)
                    yraw_tiles[g] = (yr, ecn)
            for p in pending:
                p()

            order = [g for g in range(NSQT * NNT) if g not in yraw_tiles] + \
                    [g for g in range(NSQT * NNT) if g in yraw_tiles]
            for gi_b, g in enumerate(order):
                if True:
                    i, n = divmod(g, NNT)
                    pool = psA if gi_b % 2 == 0 else psL
                    tag = "at" if pool is psA else "ly"
                    ps_y = pool.tile([128, 512], F32, tag=tag)
                    ec0 = yraw_tiles[g][1] if g in yraw_tiles else 0
                    for ec in range(ec0, H):
                        nc.tensor.matmul(
                            ps_y,
                            lhsT=attn_tiles[ec][:, i * 128:(i + 1) * 128],
                            rhs=w_sb[:, ec, n * 512:(n + 1) * 512],
                            start=(ec == ec0), stop=(ec == H - 1),
                        )
                    yt = yout.tile([128, 512], F32)
                    if gi_b >= NSQT * NNT - 2:
                        for hf in range(2):
                            sl = slice(hf * 256, (hf + 1) * 256)
                            if g in yraw_tiles:
                                nc.vector.tensor_add(
                                    yt[:, sl], ps_y[:, sl],
                                    yraw_tiles[g][0][:, sl])
                            else:
                                nc.scalar.copy(yt[:, sl], ps_y[:, sl])
                            nc.sync.dma_start(
                                y[i * 128:(i + 1) * 128,
                                  n * 512 + hf * 256:n * 512 + (hf + 1) * 256],
                                yt[:, sl],
                            )
                    else:
                        if g in yraw_tiles:
                            nc.vector.tensor_add(yt, ps_y, yraw_tiles[g][0])
                        else:
                            nc.scalar.copy(yt, ps_y)
                        nc.sync.dma_start(
                            y[i * 128:(i + 1) * 128, n * 512:(n + 1) * 512],
                            yt,
                        )

    _split_excess_waits(nc)
    return nc


_PROGRAMS = {}


def _get_program(nfull, nche):
    key = (nfull, nche)
    if key not in _PROGRAMS:
        _PROGRAMS[key] = _build_program(nfull, nche)
    return _PROGRAMS[key]


def _make_in_maps(keys, values, queries, attention_mask, w_out):
    bf = ml_dtypes.bfloat16
    wT_host = np.ascontiguousarray(w_out.astype(bf).T)

    nv = attention_mask.sum(axis=1).astype(np.int64)
    nfull = int(nv.min()) // 128
    nche = max(1, int(-(-int(nv.max()) // 128)))
    nk = nche * 128

    per_batch = []
    for b in range(B):
        order = np.argsort(~attention_mask[b], kind="stable")[:nk]
        kb = keys[b][order].astype(bf).reshape(nk, H, C)
        kT_host = np.ascontiguousarray(kb.transpose(1, 2, 0))
        vb = values[b][order].astype(bf).reshape(nche, 128, H, C)
        v_host = np.ascontiguousarray(vb.transpose(2, 1, 0, 3))
        mb = np.where(attention_mask[b][order], 0.0, MASK_NEG).astype(np.float32)
        maskb_host = np.ascontiguousarray(mb.reshape(nche, 128).T)
        per_batch.append((kT_host, v_host, maskb_host))

    in_maps = []
    for core in range(8):
        b = core // 4
        q0 = (core % 4) * SQ
        qb = queries[b, q0:q0 + SQ].astype(bf).reshape(SQ, H, C)
        qT_host = np.ascontiguousarray(qb.transpose(1, 2, 0))
        kT_host, v_host, maskb_host = per_batch[b]
        in_maps.append({
            "qT": qT_host,
            "kT": kT_host,
            "v": v_host,
            "wT": wT_host,
            "maskb": maskb_host,
        })
    return in_maps, nfull, nche


def _run(inputs, trace=False, trace_cores=None):
    from concourse.bass_utils import run_bass_kernel_spmd

    in_maps, nfull, nche = _make_in_maps(**inputs)
    nc = _get_program(nfull, nche)
    res = run_bass_kernel_spmd(
        nc, in_maps, core_ids=list(range(8)),
        trace=trace, trace_cores=trace_cores,
    )
    out = np.empty((B, S, E), dtype=np.float32)
    for core in range(8):
        b = core // 4
        q0 = (core % 4) * SQ
        out[b, q0:q0 + SQ, :] = res.results[core]["y"]
    return out, res


def kernel(keys, values, queries, attention_mask, w_out):
    out, _ = _run(dict(
        keys=np.asarray(keys), values=np.asarray(values),
        queries=np.asarray(queries),
        attention_mask=np.asarray(attention_mask),
        w_out=np.asarray(w_out),
    ))
    return out
